# revision 12
# baseline (speedup 1.0000x reference)
"""Causal multi-head attention with RoPE (faithful to reference's cos<-sin
overwrite bug) on 8 TRN2 NeuronCores.

Sharding: data parallel on batch (2) x tensor parallel on heads (4 groups of
4 heads) = 8 cores. Each core computes, for its (batch, 4 heads), flash-style
causal attention and its partial out-projection; the host sums the 4 partials
per batch and adds the bias.

Structure (v4):
- RoPE's rotate-half is folded into Wq/Wk on the host; on-device rope is one
  elementwise multiply by a sin table.
- Q/K projections run in fp8e4 MatmulPerfMode.DoubleRow (half cost): the host
  ships x^T and the folded Wq/Wk pre-packed as [128, 4, 2, .] with
  contraction element e = 256*ci + 128*j + p.
- Scores are computed transposed (S^T[k, q]) per 128-row k-chunk over 512-wide
  q supertiles, BOTH heads of a pair into one [128, 2, 512] PSUM tile; exp
  runs once per chunk over the strided live region of both heads; a
  triangular mask multiply covers the diagonal 128-block of both heads.
  Score matmuls are fp8 DoubleRow on [32, 2, ctx]-packed q/k (DMA-repacked
  from the rope output).
- A/V runs in output-partition form: per 128-q tile, O[q, d] accumulates in
  PSUM over k-chunks with pt as the stationary operand; an extra ones column
  in V yields softmax row-sums in the same pass. Normalization is a DVE
  reciprocal ([128,1]) + per-partition tensor_scalar multiply on eviction.
- Normalized O tiles are pair-transposed on the PE (via identity) into [d, q]
  layout for the out-projection; z is staged in SBUF and written to DRAM two
  128-row tiles per DMA (last two tiles individually to shorten the tail).
- Passes are ordered (supertile, pair) ascending so per-chunk PE work grows
  as filler work (V projection, later-supertile Q/K projections) runs out.
"""

import contextlib

import numpy as np
import ml_dtypes

import concourse.bass as bass
import concourse.mybir as mybir
import concourse.tile as tile
from concourse.bass_utils import run_bass_kernel_spmd

BATCH, CTX, ED = 2, 2048, 1024
NH, HD = 16, 64
ROPE_BASE = 10000.0
P = 128
NCORES = 8
HPC = 4  # heads per core
SS = 512  # q supertile width
NJ = CTX // SS  # 4 supertiles
NKT = CTX // P  # 16 k-chunks / ctx tiles
NEC = ED // P  # 8

F32 = mybir.dt.float32
BF16 = mybir.dt.bfloat16
FP8 = mybir.dt.float8e4


def _split_multi_waits(nc, max_waits=1):
    """Walrus in this container rejects >1 sync wait per instruction; hoist
    extra waits onto preceding same-engine NoOps (semantically identical:
    engines execute their stream in order)."""
    n = 0
    for func in nc.m.functions:
        for bb in func.blocks:
            insts = list(bb.instructions)
            out = []
            changed = False
            for inst in insts:
                si = inst.sync_info
                if si and si.on_wait and len(si.on_wait) > max_waits:
                    waits = list(si.on_wait)
                    for k, w in enumerate(waits[:-max_waits]):
                        nop = mybir.InstNoOp(
                            name=f"{inst.name}-ws{k}",
                            sync_info=mybir.SyncInfo(on_wait=[w], on_update=[]),
                        )
                        nop.engine = inst.engine
                        out.append(nop)
                        n += 1
                    inst.sync_info = mybir.SyncInfo(
                        on_wait=waits[-max_waits:], on_update=list(si.on_update or [])
                    )
                    changed = True
                out.append(inst)
            if changed:
                bb.instructions = out
    return n


def _emit(nc, xT, x8, w8q, w8k, wv, wo, sin2, tri2, ident, z, tc):
    Exp = mybir.ActivationFunctionType.Exp
    MULT = mybir.AluOpType.mult
    DR = mybir.MatmulPerfMode.DoubleRow

    with contextlib.ExitStack() as ctx:
        pers = ctx.enter_context(tc.tile_pool(name="pers", bufs=1))
        ptp = ctx.enter_context(tc.tile_pool(name="ptp", bufs=18))
        work = ctx.enter_context(tc.tile_pool(name="work", bufs=2))
        psum = ctx.enter_context(tc.tile_pool(name="psum", bufs=1, space="PSUM"))

        xt_sb = pers.tile([P, NEC, CTX], BF16, tag="xt")
        x8_sb = pers.tile([P, 4, 2, CTX], FP8, tag="x8")
        w8q_sb = pers.tile([P, 4, 2, 256], FP8, tag="w8q")
        w8k_sb = pers.tile([P, 4, 2, 256], FP8, tag="w8k")
        wv_sb = pers.tile([P, NEC, 256], BF16, tag="wv")
        wo_sb = pers.tile([P, 2, ED], BF16, tag="wo")
        sin_sb = pers.tile([P, CTX], BF16, tag="sin")
        tri_sb = pers.tile([P, 2, P], BF16, tag="tri")
        id_sb = pers.tile([P, P], BF16, tag="id")
        v_sb = pers.tile([P, NKT, HPC, 66], BF16, tag="v")
        ot_sb = pers.tile([P, 2, CTX], BF16, tag="ot")
        qt0 = pers.tile([P, 2, SS], BF16, tag="qt0")
        kt0 = pers.tile([P, 2, SS], BF16, tag="kt0")
        q8p = pers.tile([P, 2, CTX], FP8, tag="q8p")
        k8p = pers.tile([P, 2, CTX], FP8, tag="k8p")
        q8 = pers.tile([32, HPC, 2, CTX], FP8, tag="q8")
        k8 = pers.tile([32, HPC, 2, CTX], FP8, tag="k8")

        def ld(dst, src):
            nc.sync.dma_start(dst, src)

        # ---- load batch A: what the first two supertiles' projections and
        # the early V projections need; the rest loads after the early
        # relayout DMAs are emitted (SP processes its stream in order) ----
        ld(w8q_sb[:], w8q)
        ld(w8k_sb[:], w8k)
        ld(x8_sb[:, :, :, 0:512], x8[:, :, :, 0:512])
        ld(sin_sb[:, 0:1024], sin2[:, 0:1024])
        ld(x8_sb[:, :, :, 512:1024], x8[:, :, :, 512:1024])
        ld(xt_sb[:, 0:4, 0:512], xT[0:512, 0:512].rearrange("(c p) n -> p c n", p=P))
        ld(xt_sb[:, 4:8, 0:512], xT[512:1024, 0:512].rearrange("(c p) n -> p c n", p=P))
        ld(tri_sb[:], tri2)
        ld(id_sb[:], ident)
        for c2 in range(2):
            ld(
                wv_sb[:, 4 * c2 : 4 * c2 + 4, :],
                wv[512 * c2 : 512 * (c2 + 1), :].rearrange("(c p) n -> p c n", p=P),
            )
        ld(wo_sb[:], wo.rearrange("(cc p) n -> p cc n", p=P))
        nc.gpsimd.memset(v_sb[:, :, :, 64:65], 1.0)

        def load_b1():
            ld(
                xt_sb[:, :, 512:1024],
                xT[:, 512:1024].rearrange("(c p) n -> p c n", p=P),
            )

        def load_b2():
            ld(sin_sb[:, 1024:2048], sin2[:, 1024:2048])
            ld(x8_sb[:, :, :, 1024:2048], x8[:, :, :, 1024:2048])
            ld(
                xt_sb[:, :, 1024:1536],
                xT[:, 1024:1536].rearrange("(c p) n -> p c n", p=P),
            )

        def load_b3():
            ld(
                xt_sb[:, :, 1536:2048],
                xT[:, 1536:2048].rearrange("(c p) n -> p c n", p=P),
            )

        # ---- Q/K projection (fp8 DoubleRow) + rope for one (which, pair,
        # supertile); relayout DMAs repack rope output into [32, 2, .] ----
        def qk_proj(which, p, j5, ptag="aux"):
            w8 = w8q_sb if which == "q" else w8k_sb
            if ptag == "st":
                ps = psum.tile([P, 2, SS], F32, tag="st", bufs=2, name="psqk")
                psv = ps[:, 0, :]
            else:
                ps = psum.tile([P, ED], F32, tag="aux", bufs=1, name="psqk")
                psv = ps[:, 0:SS]
            c0 = j5 * SS
            for ci in range(4):
                nc.tensor.matmul(
                    psv,
                    lhsT=w8[:, ci, :, p * P : (p + 1) * P],
                    rhs=x8_sb[:, ci, :, c0 : c0 + SS],
                    start=(ci == 0),
                    stop=(ci == 3),
                    perf_mode=DR,
                )
            if j5 == 0:
                # supertile-0 scores run in bf16 straight off the rope
                # output; k is cast+repacked later, off the critical path
                dst = qt0 if which == "q" else kt0
                nc.vector.tensor_tensor(
                    out=dst[:, p, :], in0=psv, in1=sin_sb[:, c0 : c0 + SS], op=MULT
                )
                return
            dst = q8p if which == "q" else k8p
            nc.vector.tensor_tensor(
                out=dst[:, p, c0 : c0 + SS],
                in0=psv,
                in1=sin_sb[:, c0 : c0 + SS],
                op=MULT,
            )
            # repack for DoubleRow scores: j5 1 at 512 granularity (feeds
            # the early passes sooner), j5 2/3 as one 1024 block on j5=3
            src, pk = (q8p, q8) if which == "q" else (k8p, k8)

            def relayout(a, b):
                for s in range(2):
                    for jj in range(2):
                        ld(
                            pk[:, 2 * p + s, jj, a:b],
                            src[s * HD + 32 * jj : s * HD + 32 * jj + 32, p, a:b],
                        )

            if j5 == 1:
                relayout(SS, 2 * SS)
            elif j5 == 3:
                relayout(2 * SS, 4 * SS)

        # deferred: cast supertile-0 k to fp8 and repack (needed by fp8
        # scores of supertiles 1..3, k-chunks 0..3)
        def cast_relayout_k_j0(p):
            nc.vector.tensor_copy(out=k8p[:, p, 0:SS], in_=kt0[:, p, :])
            for s in range(2):
                for jj in range(2):
                    ld(
                        k8[:, 2 * p + s, jj, 0:SS],
                        k8p[s * HD + 32 * jj : s * HD + 32 * jj + 32, p, 0:SS],
                    )

        # ---- V projection (bf16, natural layout, per ctx tile) ----
        def v_proj(t):
            ps = psum.tile([P, 256], F32, tag="small", bufs=2, name="psv")
            for c in range(NEC):
                nc.tensor.matmul(
                    ps[:],
                    lhsT=xt_sb[:, c, t * P : (t + 1) * P],
                    rhs=wv_sb[:, c, :],
                    start=(c == 0),
                    stop=(c == NEC - 1),
                )
            nc.vector.tensor_copy(
                out=v_sb[:, t, :, 0:64],
                in_=ps[:].rearrange("p (h d) -> p h d", h=HPC),
            )

        # ---- scores+exp(+mask) for both heads of (pair, supertile, chunk) --
        def scores(p, j5, KT):
            st = psum.tile([P, 2, SS], F32, tag="st", bufs=2)
            lo = max(KT * P - j5 * SS, 0)
            for s in range(2):
                h = 2 * p + s
                if j5 == 0:
                    nc.tensor.matmul(
                        st[:, s, lo:SS],
                        lhsT=kt0[s * HD : (s + 1) * HD, p, KT * P : (KT + 1) * P],
                        rhs=qt0[s * HD : (s + 1) * HD, p, lo:SS],
                        start=True,
                        stop=True,
                    )
                else:
                    nc.tensor.matmul(
                        st[:, s, lo:SS],
                        lhsT=k8[:, h, :, KT * P : (KT + 1) * P],
                        rhs=q8[:, h, :, j5 * SS + lo : (j5 + 1) * SS],
                        start=True,
                        stop=True,
                        perf_mode=DR,
                    )
            pt = ptp.tile([P, 2, SS], BF16, tag="pt")
            nc.scalar.activation(pt[:, :, lo:SS], st[:, :, lo:SS], Exp, scale=0.125)
            if KT >= 4 * j5:
                d = KT * P - j5 * SS
                nc.vector.tensor_tensor(
                    out=pt[:, :, d : d + P],
                    in0=pt[:, :, d : d + P],
                    in1=tri_sb[:],
                    op=MULT,
                )
            return pt

        # ---- A/V for one (head, supertile, local q tile) in O[q,d] form ----
        def av(h, j5, tl, pts, osb_t):
            T = 4 * j5 + tl
            s = h % 2
            o = psum.tile([P, 66], F32, tag="small", bufs=2)
            for KT in range(T + 1):
                nc.tensor.matmul(
                    o[:, 0:65],
                    lhsT=pts[KT][:, s, tl * P : (tl + 1) * P],
                    rhs=v_sb[:, KT, h, 0:65],
                    start=(KT == 0),
                    stop=(KT == T),
                )
            rc = work.tile([P, 1], F32, tag="rc", bufs=6)
            nc.vector.reciprocal(rc[:], o[:, 64:65])
            if j5 <= 1:
                # early passes: DVE carries the next pass's rope on its
                # in-order queue; evict on the under-used ACT instead
                nc.scalar.activation(
                    osb_t[:, h, :], o[:, 0:64],
                    mybir.ActivationFunctionType.Copy, scale=rc[:],
                )
            else:
                nc.vector.tensor_scalar(
                    out=osb_t[:, h, :],
                    in0=o[:, 0:64],
                    scalar1=rc[:],
                    scalar2=None,
                    op0=MULT,
                )

        # ---- pair transpose of normalized O into [d, q] for out_proj ----
        def transpose_pair(cc, T, osb_t):
            tr = psum.tile([P, P], BF16, tag="small", bufs=2)
            nc.tensor.transpose(tr[:], osb_t[:, 2 * cc : 2 * cc + 2, :], id_sb[:])
            nc.vector.tensor_copy(out=ot_sb[:, cc, T * P : (T + 1) * P], in_=tr[:])

        # ---- out projection; z staged 2 tiles per DMA (last two single) ----
        zstage = {}

        def out_proj(T):
            zp = psum.tile([P, ED], F32, tag="aux", bufs=1)
            for cc in (0, 1):
                for nh in (0, 1):
                    nc.tensor.matmul(
                        zp[:, nh * 512 : (nh + 1) * 512],
                        lhsT=ot_sb[:, cc, T * P : (T + 1) * P],
                        rhs=wo_sb[:, cc, nh * 512 : (nh + 1) * 512],
                        start=(cc == 0),
                        stop=(cc == 1),
                    )
            if T % 2 == 0:
                zstage[T // 2] = work.tile([P, 2, ED], F32, tag="zs", bufs=2, name=f"zs{T}")
            zs_t = zstage[T // 2]
            if T == NKT - 1:
                nc.vector.tensor_copy(out=zs_t[:, 1, 0:512], in_=zp[:, 0:512])
                nc.scalar.activation(
                    zs_t[:, 1, 512:1024], zp[:, 512:1024],
                    mybir.ActivationFunctionType.Copy,
                )
                ld(z[T * P : (T + 1) * P, :], zs_t[:, 1, :])
            elif T == NKT - 2:
                nc.vector.tensor_copy(out=zs_t[:, 0, :], in_=zp[:])
                ld(z[T * P : (T + 1) * P, :], zs_t[:, 0, :])
            else:
                nc.vector.tensor_copy(out=zs_t[:, T % 2, :], in_=zp[:])
                if T % 2 == 1:
                    ld(
                        z[(T - 1) * P : (T + 1) * P, :].rearrange(
                            "(a p) n -> p a n", p=P
                        ),
                        zs_t[:],
                    )

        # ---- emission schedule: passes (j5 ascending, pair inner) ----
        fillers = []

        def pull_filler():
            if fillers:
                fillers.pop(0)()

        # pre-phase: pair0/supertile0 projections on the idle scores psum
        qk_proj("q", 0, 0, ptag="st")
        qk_proj("k", 0, 0, ptag="st")
        # filler queue: each unit needed well before its consuming pass
        fillers.append(lambda: cast_relayout_k_j0(0))
        fillers.append(lambda: qk_proj("q", 1, 0))
        fillers.append(lambda: qk_proj("k", 1, 0))
        fillers.extend([(lambda t=t: v_proj(t)) for t in (2, 3)])
        fillers.append(lambda: qk_proj("q", 0, 1))
        fillers.append(lambda: qk_proj("k", 0, 1))
        fillers.append(load_b1)
        fillers.append(lambda: cast_relayout_k_j0(1))
        fillers.extend([(lambda t=t: v_proj(t)) for t in (4, 5)])
        fillers.append(lambda: qk_proj("q", 1, 1))
        fillers.append(lambda: qk_proj("k", 1, 1))
        fillers.extend([(lambda t=t: v_proj(t)) for t in (6, 7)])
        fillers.append(load_b2)
        fillers.append(lambda: qk_proj("q", 0, 2))
        fillers.append(lambda: qk_proj("k", 0, 2))
        fillers.append(lambda: qk_proj("q", 0, 3))
        fillers.append(lambda: qk_proj("k", 0, 3))
        fillers.append(load_b3)
        fillers.append(lambda: qk_proj("q", 1, 2))
        fillers.append(lambda: qk_proj("k", 1, 2))
        fillers.append(lambda: qk_proj("q", 1, 3))
        fillers.append(lambda: qk_proj("k", 1, 3))
        fillers.extend([(lambda t=t: v_proj(t)) for t in range(8, 16)])

        # v0/v1 are needed by the very first avs: emit directly
        v_proj(0)
        v_proj(1)

        osb = {}
        for j5 in range(NJ):
            for p in (0, 1):
                pts = {}
                for KT in range(4 * (j5 + 1)):
                    pull_filler()
                    pts[KT] = scores(p, j5, KT)
                    tl = KT - 4 * j5
                    if tl >= 0:
                        T = KT
                        if p == 0:
                            osb[T] = work.tile(
                                [P, HPC, 64], BF16, tag="osb", bufs=8, name=f"osb{T}"
                            )
                        av(2 * p, j5, tl, pts, osb[T])
                        av(2 * p + 1, j5, tl, pts, osb[T])
                        if p == 1:
                            transpose_pair(0, T, osb[T])
                            transpose_pair(1, T, osb[T])
                            out_proj(T)
        while fillers:
            pull_filler()


def _build_program(split_waits=True):
    nc = bass.Bass("TRN2", target_bir_lowering=False, debug=False, num_devices=NCORES)
    xT = nc.dram_tensor("xT", [ED, CTX], BF16, kind="ExternalInput").ap()
    x8 = nc.dram_tensor("x8", [P, 4, 2, CTX], FP8, kind="ExternalInput").ap()
    w8q = nc.dram_tensor("w8q", [P, 4, 2, 256], FP8, kind="ExternalInput").ap()
    w8k = nc.dram_tensor("w8k", [P, 4, 2, 256], FP8, kind="ExternalInput").ap()
    wv = nc.dram_tensor("wv", [ED, 256], BF16, kind="ExternalInput").ap()
    wo = nc.dram_tensor("wo", [256, ED], BF16, kind="ExternalInput").ap()
    sin2 = nc.dram_tensor("sin2", [P, CTX], BF16, kind="ExternalInput").ap()
    tri2 = nc.dram_tensor("tri2", [P, 2 * P], BF16, kind="ExternalInput").ap()
    ident = nc.dram_tensor("ident", [P, P], BF16, kind="ExternalInput").ap()
    z = nc.dram_tensor("z", [CTX, ED], F32, kind="ExternalOutput").ap()
    with tile.TileContext(nc) as tc:
        _emit(nc, xT, x8, w8q, w8k, wv, wo, sin2, tri2, ident, z, tc)
    if split_waits:
        _split_multi_waits(nc)
    return nc


_PROGRAM = None


def _get_program():
    global _PROGRAM
    if _PROGRAM is None:
        _PROGRAM = _build_program()
    return _PROGRAM


def _host_tables():
    # rotate-half fold matrix: q_rot = R q
    Rm = np.zeros((HD, HD), np.float32)
    for i in range(HD // 2):
        Rm[i, i] = 1.0
        Rm[i, i + 32] = -1.0
        Rm[i + 32, i + 32] = 1.0
        Rm[i + 32, i] = 1.0
    j = np.arange(HD // 2, dtype=np.float32)
    thetas = 1.0 / ROPE_BASE ** (2.0 * j / (HD // 2))
    pos = np.arange(CTX, dtype=np.float32)
    ang = pos[:, None] * thetas[None, :]
    sinT = np.sin(np.concatenate([ang, ang], axis=-1)).T.astype(np.float32)  # [64,CTX]
    sin2 = np.ascontiguousarray(np.tile(sinT, (2, 1))).astype(
        ml_dtypes.bfloat16
    )  # [128, CTX]
    cg = np.arange(P)[None, :]
    ii = np.arange(P)[:, None]
    tri = (cg >= ii).astype(np.float32)  # keep q >= k
    tri2 = np.ascontiguousarray(np.concatenate([tri, tri], axis=1)).astype(
        ml_dtypes.bfloat16
    )  # [128, 256]
    ident = np.eye(P, dtype=np.float32).astype(ml_dtypes.bfloat16)
    return Rm, sin2, tri2, ident


def _pack_dr(a):
    """[1024, n] -> [128, 4, 2, n] fp8 with element e = 256*ci + 128*j + p."""
    n = a.shape[1]
    return np.ascontiguousarray(
        a.reshape(4, 2, P, n).transpose(2, 0, 1, 3)
    ).astype(ml_dtypes.float8_e4m3)


def _run(x, Wq, Wk, Wv, Wo):
    nc = _get_program()
    Rm, sin2, tri2, ident = _host_tables()

    def fold(W):
        W2 = W.reshape(ED, NH, HD)
        return np.einsum("enh,gh->eng", W2, Rm).reshape(ED, NH * HD)

    bf = ml_dtypes.bfloat16
    Wq_f = fold(Wq)
    Wk_f = fold(Wk)
    Wv_b = Wv.astype(bf)
    Wo_b = Wo.astype(bf)
    xT_f = [np.ascontiguousarray(x[b].T) for b in range(BATCH)]
    x8_b = [_pack_dr(t) for t in xT_f]
    xT_b = [t.astype(bf) for t in xT_f]

    in_maps = []
    for core in range(NCORES):
        b, g = core // 4, core % 4
        cs = slice(256 * g, 256 * (g + 1))
        in_maps.append(
            {
                "xT": xT_b[b],
                "x8": x8_b[b],
                "w8q": _pack_dr(np.ascontiguousarray(Wq_f[:, cs])),
                "w8k": _pack_dr(np.ascontiguousarray(Wk_f[:, cs])),
                "wv": np.ascontiguousarray(Wv_b[:, cs]),
                "wo": np.ascontiguousarray(Wo_b[cs, :]),
                "sin2": sin2,
                "tri2": tri2,
                "ident": ident,
            }
        )
    return nc, in_maps


def kernel(x, Wq, Wk, Wv, Wo, bo):
    x = np.asarray(x, dtype=np.float32)
    nc, in_maps = _run(x, np.asarray(Wq, np.float32), np.asarray(Wk, np.float32),
                       np.asarray(Wv, np.float32), np.asarray(Wo, np.float32))
    res = run_bass_kernel_spmd(nc, in_maps, core_ids=list(range(NCORES)))
    out = np.zeros((BATCH, CTX, ED), np.float32)
    for core in range(NCORES):
        b = core // 4
        out[b] += res.results[core]["z"]
    out += np.asarray(bo, np.float32)[None, None, :]
    return out


# revision 13
# speedup vs baseline: 1.0118x; 1.0118x over previous
"""Causal multi-head attention with RoPE (faithful to reference's cos<-sin
overwrite bug) on 8 TRN2 NeuronCores.

Sharding: data parallel on batch (2) x tensor parallel on heads (4 groups of
4 heads) = 8 cores. Each core computes, for its (batch, 4 heads), flash-style
causal attention and its partial out-projection; the host sums the 4 partials
per batch and adds the bias.

Structure (v4):
- RoPE's rotate-half is folded into Wq/Wk on the host; on-device rope is one
  elementwise multiply by a sin table.
- Q/K projections run in fp8e4 MatmulPerfMode.DoubleRow (half cost): the host
  ships x^T and the folded Wq/Wk pre-packed as [128, 4, 2, .] with
  contraction element e = 256*ci + 128*j + p.
- Scores are computed transposed (S^T[k, q]) per 128-row k-chunk over 512-wide
  q supertiles, BOTH heads of a pair into one [128, 2, 512] PSUM tile; exp
  runs once per chunk over the strided live region of both heads; a
  triangular mask multiply covers the diagonal 128-block of both heads.
  Score matmuls are fp8 DoubleRow on [32, 2, ctx]-packed q/k (DMA-repacked
  from the rope output).
- A/V runs in output-partition form: per 128-q tile, O[q, d] accumulates in
  PSUM over k-chunks with pt as the stationary operand; an extra ones column
  in V yields softmax row-sums in the same pass. Normalization is a DVE
  reciprocal ([128,1]) + per-partition tensor_scalar multiply on eviction.
- Normalized O tiles are pair-transposed on the PE (via identity) into [d, q]
  layout for the out-projection; z is staged in SBUF and written to DRAM two
  128-row tiles per DMA (last two tiles individually to shorten the tail).
- Passes are ordered (supertile, pair) ascending so per-chunk PE work grows
  as filler work (V projection, later-supertile Q/K projections) runs out.
"""

import contextlib

import numpy as np
import ml_dtypes

import concourse.bass as bass
import concourse.mybir as mybir
import concourse.tile as tile
from concourse.bass_utils import run_bass_kernel_spmd

BATCH, CTX, ED = 2, 2048, 1024
NH, HD = 16, 64
ROPE_BASE = 10000.0
P = 128
NCORES = 8
HPC = 4  # heads per core
SS = 512  # q supertile width
NJ = CTX // SS  # 4 supertiles
NKT = CTX // P  # 16 k-chunks / ctx tiles
NEC = ED // P  # 8

F32 = mybir.dt.float32
BF16 = mybir.dt.bfloat16
FP8 = mybir.dt.float8e4


def _split_multi_waits(nc, max_waits=1):
    """Walrus in this container rejects >1 sync wait per instruction; hoist
    extra waits onto preceding same-engine NoOps (semantically identical:
    engines execute their stream in order)."""
    n = 0
    for func in nc.m.functions:
        for bb in func.blocks:
            insts = list(bb.instructions)
            out = []
            changed = False
            for inst in insts:
                si = inst.sync_info
                if si and si.on_wait and len(si.on_wait) > max_waits:
                    waits = list(si.on_wait)
                    for k, w in enumerate(waits[:-max_waits]):
                        nop = mybir.InstNoOp(
                            name=f"{inst.name}-ws{k}",
                            sync_info=mybir.SyncInfo(on_wait=[w], on_update=[]),
                        )
                        nop.engine = inst.engine
                        out.append(nop)
                        n += 1
                    inst.sync_info = mybir.SyncInfo(
                        on_wait=waits[-max_waits:], on_update=list(si.on_update or [])
                    )
                    changed = True
                out.append(inst)
            if changed:
                bb.instructions = out
    return n


def _emit(nc, xT, x8, w8q, w8k, wv, wo, sin2, tri2, ident, z, tc):
    Exp = mybir.ActivationFunctionType.Exp
    MULT = mybir.AluOpType.mult
    DR = mybir.MatmulPerfMode.DoubleRow

    with contextlib.ExitStack() as ctx:
        pers = ctx.enter_context(tc.tile_pool(name="pers", bufs=1))
        ptp = ctx.enter_context(tc.tile_pool(name="ptp", bufs=18))
        work = ctx.enter_context(tc.tile_pool(name="work", bufs=2))
        psum = ctx.enter_context(tc.tile_pool(name="psum", bufs=1, space="PSUM"))

        xt_sb = pers.tile([P, NEC, CTX], BF16, tag="xt")
        x8_sb = pers.tile([P, 4, 2, CTX], FP8, tag="x8")
        w8q_sb = pers.tile([P, 4, 2, 256], FP8, tag="w8q")
        w8k_sb = pers.tile([P, 4, 2, 256], FP8, tag="w8k")
        wv_sb = pers.tile([P, NEC, 256], BF16, tag="wv")
        wo_sb = pers.tile([P, 2, ED], BF16, tag="wo")
        sin_sb = pers.tile([P, CTX], BF16, tag="sin")
        tri_sb = pers.tile([P, 2, P], BF16, tag="tri")
        id_sb = pers.tile([P, P], BF16, tag="id")
        v_sb = pers.tile([P, NKT, HPC, 66], BF16, tag="v")
        ot_sb = pers.tile([P, 2, CTX], BF16, tag="ot")
        qt0 = pers.tile([P, 2, SS], BF16, tag="qt0")
        kt0 = pers.tile([P, 2, SS], BF16, tag="kt0")
        q8p = pers.tile([P, 2, CTX], FP8, tag="q8p")
        k8p = pers.tile([P, 2, CTX], FP8, tag="k8p")
        q8 = pers.tile([32, HPC, 2, CTX], FP8, tag="q8")
        k8 = pers.tile([32, HPC, 2, CTX], FP8, tag="k8")

        def ld(dst, src):
            nc.sync.dma_start(dst, src)

        # ---- load batch A: what the first two supertiles' projections and
        # the early V projections need; the rest loads after the early
        # relayout DMAs are emitted (SP processes its stream in order) ----
        ld(w8q_sb[:], w8q)
        ld(w8k_sb[:], w8k)
        ld(x8_sb[:, :, :, 0:512], x8[:, :, :, 0:512])
        ld(sin_sb[:, 0:1024], sin2[:, 0:1024])
        ld(x8_sb[:, :, :, 512:1024], x8[:, :, :, 512:1024])
        ld(xt_sb[:, 0:4, 0:512], xT[0:512, 0:512].rearrange("(c p) n -> p c n", p=P))
        ld(xt_sb[:, 4:8, 0:512], xT[512:1024, 0:512].rearrange("(c p) n -> p c n", p=P))
        ld(tri_sb[:], tri2)
        ld(id_sb[:], ident)
        for c2 in range(2):
            ld(
                wv_sb[:, 4 * c2 : 4 * c2 + 4, :],
                wv[512 * c2 : 512 * (c2 + 1), :].rearrange("(c p) n -> p c n", p=P),
            )
        ld(wo_sb[:], wo.rearrange("(cc p) n -> p cc n", p=P))
        nc.gpsimd.memset(v_sb[:, :, :, 64:65], 1.0)

        def load_b1():
            ld(
                xt_sb[:, :, 512:1024],
                xT[:, 512:1024].rearrange("(c p) n -> p c n", p=P),
            )

        def load_b2():
            ld(sin_sb[:, 1024:2048], sin2[:, 1024:2048])
            ld(x8_sb[:, :, :, 1024:2048], x8[:, :, :, 1024:2048])
            ld(
                xt_sb[:, :, 1024:1536],
                xT[:, 1024:1536].rearrange("(c p) n -> p c n", p=P),
            )

        def load_b3():
            ld(
                xt_sb[:, :, 1536:2048],
                xT[:, 1536:2048].rearrange("(c p) n -> p c n", p=P),
            )

        # ---- Q/K projection (fp8 DoubleRow) + rope for one (which, pair,
        # supertile); relayout DMAs repack rope output into [32, 2, .] ----
        def qk_proj(which, p, j5, ptag="aux"):
            w8 = w8q_sb if which == "q" else w8k_sb
            if ptag == "st":
                ps = psum.tile([P, 2, SS], F32, tag="st", bufs=2, name="psqk")
                psv = ps[:, 0, :]
            else:
                ps = psum.tile([P, ED], F32, tag="aux", bufs=1, name="psqk")
                psv = ps[:, 0:SS]
            c0 = j5 * SS
            for ci in range(4):
                nc.tensor.matmul(
                    psv,
                    lhsT=w8[:, ci, :, p * P : (p + 1) * P],
                    rhs=x8_sb[:, ci, :, c0 : c0 + SS],
                    start=(ci == 0),
                    stop=(ci == 3),
                    perf_mode=DR,
                )
            if j5 == 0:
                # supertile-0 scores run in bf16 straight off the rope
                # output; k is cast+repacked later, off the critical path
                dst = qt0 if which == "q" else kt0
                nc.vector.tensor_tensor(
                    out=dst[:, p, :], in0=psv, in1=sin_sb[:, c0 : c0 + SS], op=MULT
                )
                return
            dst = q8p if which == "q" else k8p
            nc.vector.tensor_tensor(
                out=dst[:, p, c0 : c0 + SS],
                in0=psv,
                in1=sin_sb[:, c0 : c0 + SS],
                op=MULT,
            )
            # repack for DoubleRow scores: j5 1 at 512 granularity (feeds
            # the early passes sooner), j5 2/3 as one 1024 block on j5=3
            src, pk = (q8p, q8) if which == "q" else (k8p, k8)

            def relayout(a, b):
                for s in range(2):
                    for jj in range(2):
                        ld(
                            pk[:, 2 * p + s, jj, a:b],
                            src[s * HD + 32 * jj : s * HD + 32 * jj + 32, p, a:b],
                        )

            if j5 == 1:
                relayout(SS, 2 * SS)
            elif j5 == 3:
                relayout(2 * SS, 4 * SS)

        # deferred: cast supertile-0 k to fp8 and repack (needed by fp8
        # scores of supertiles 1..3, k-chunks 0..3)
        def cast_relayout_k_j0(p):
            nc.vector.tensor_copy(out=k8p[:, p, 0:SS], in_=kt0[:, p, :])
            for s in range(2):
                for jj in range(2):
                    ld(
                        k8[:, 2 * p + s, jj, 0:SS],
                        k8p[s * HD + 32 * jj : s * HD + 32 * jj + 32, p, 0:SS],
                    )

        # ---- V projection (bf16, natural layout, per ctx tile) ----
        def v_proj(t):
            ps = psum.tile([P, 256], F32, tag="small", bufs=2, name="psv")
            for c in range(NEC):
                nc.tensor.matmul(
                    ps[:],
                    lhsT=xt_sb[:, c, t * P : (t + 1) * P],
                    rhs=wv_sb[:, c, :],
                    start=(c == 0),
                    stop=(c == NEC - 1),
                )
            nc.vector.tensor_copy(
                out=v_sb[:, t, :, 0:64],
                in_=ps[:].rearrange("p (h d) -> p h d", h=HPC),
            )

        # ---- scores+exp(+mask) for both heads of (pair, supertile, chunk) --
        def scores(p, j5, KT):
            st = psum.tile([P, 2, SS], F32, tag="st", bufs=2)
            lo = max(KT * P - j5 * SS, 0)
            for s in range(2):
                h = 2 * p + s
                if j5 == 0:
                    nc.tensor.matmul(
                        st[:, s, lo:SS],
                        lhsT=kt0[s * HD : (s + 1) * HD, p, KT * P : (KT + 1) * P],
                        rhs=qt0[s * HD : (s + 1) * HD, p, lo:SS],
                        start=True,
                        stop=True,
                    )
                else:
                    nc.tensor.matmul(
                        st[:, s, lo:SS],
                        lhsT=k8[:, h, :, KT * P : (KT + 1) * P],
                        rhs=q8[:, h, :, j5 * SS + lo : (j5 + 1) * SS],
                        start=True,
                        stop=True,
                        perf_mode=DR,
                    )
            pt = ptp.tile([P, 2, SS], BF16, tag="pt")
            nc.scalar.activation(pt[:, :, lo:SS], st[:, :, lo:SS], Exp, scale=0.125)
            if KT >= 4 * j5:
                d = KT * P - j5 * SS
                nc.vector.tensor_tensor(
                    out=pt[:, :, d : d + P],
                    in0=pt[:, :, d : d + P],
                    in1=tri_sb[:],
                    op=MULT,
                )
            return pt

        # ---- A/V for one (head, supertile, local q tile) in O[q,d] form ----
        def av(h, j5, tl, pts, osb_t):
            T = 4 * j5 + tl
            s = h % 2
            o = psum.tile([P, 66], F32, tag="small", bufs=2)
            for KT in range(T + 1):
                nc.tensor.matmul(
                    o[:, 0:65],
                    lhsT=pts[KT][:, s, tl * P : (tl + 1) * P],
                    rhs=v_sb[:, KT, h, 0:65],
                    start=(KT == 0),
                    stop=(KT == T),
                )
            rc = work.tile([P, 1], F32, tag="rc", bufs=6)
            nc.vector.reciprocal(rc[:], o[:, 64:65])
            nc.vector.tensor_scalar(
                out=osb_t[:, h, :],
                in0=o[:, 0:64],
                scalar1=rc[:],
                scalar2=None,
                op0=MULT,
            )

        # ---- pair transpose of normalized O into [d, q] for out_proj ----
        def transpose_pair(cc, T, osb_t):
            tr = psum.tile([P, P], BF16, tag="small", bufs=2)
            nc.tensor.transpose(tr[:], osb_t[:, 2 * cc : 2 * cc + 2, :], id_sb[:])
            nc.vector.tensor_copy(out=ot_sb[:, cc, T * P : (T + 1) * P], in_=tr[:])

        # ---- out projection; z staged 2 tiles per DMA (last two single) ----
        zstage = {}

        def out_proj(T):
            zp = psum.tile([P, ED], F32, tag="aux", bufs=1)
            for cc in (0, 1):
                for nh in (0, 1):
                    nc.tensor.matmul(
                        zp[:, nh * 512 : (nh + 1) * 512],
                        lhsT=ot_sb[:, cc, T * P : (T + 1) * P],
                        rhs=wo_sb[:, cc, nh * 512 : (nh + 1) * 512],
                        start=(cc == 0),
                        stop=(cc == 1),
                    )
            if T % 2 == 0:
                zstage[T // 2] = work.tile([P, 2, ED], F32, tag="zs", bufs=2, name=f"zs{T}")
            zs_t = zstage[T // 2]
            if T == NKT - 1:
                nc.vector.tensor_copy(out=zs_t[:, 1, 0:512], in_=zp[:, 0:512])
                nc.scalar.activation(
                    zs_t[:, 1, 512:1024], zp[:, 512:1024],
                    mybir.ActivationFunctionType.Copy,
                )
                ld(z[T * P : (T + 1) * P, :], zs_t[:, 1, :])
            elif T == NKT - 2:
                nc.vector.tensor_copy(out=zs_t[:, 0, :], in_=zp[:])
                ld(z[T * P : (T + 1) * P, :], zs_t[:, 0, :])
            else:
                nc.vector.tensor_copy(out=zs_t[:, T % 2, :], in_=zp[:])
                if T % 2 == 1:
                    ld(
                        z[(T - 1) * P : (T + 1) * P, :].rearrange(
                            "(a p) n -> p a n", p=P
                        ),
                        zs_t[:],
                    )

        # ---- emission schedule: passes (j5 ascending, pair inner) ----
        fillers = []

        def pull_filler():
            if fillers:
                fillers.pop(0)()

        # pre-phase: pair0/supertile0 projections on the idle scores psum
        qk_proj("q", 0, 0, ptag="st")
        qk_proj("k", 0, 0, ptag="st")
        # per-pass: pre-units run at pass start (empty DVE queue for their
        # ropes); fillers are pulled one per chunk inside the pass
        pre_units = {
            (0, 0): [lambda: qk_proj("q", 1, 0), lambda: qk_proj("k", 1, 0)],
            (0, 1): [lambda: qk_proj("q", 0, 1), lambda: qk_proj("k", 0, 1)],
            (1, 0): [lambda: qk_proj("q", 1, 1), lambda: qk_proj("k", 1, 1)],
            (1, 1): [lambda: qk_proj("q", 0, 2), lambda: qk_proj("k", 0, 2),
                     lambda: qk_proj("q", 0, 3), lambda: qk_proj("k", 0, 3)],
            (2, 0): [lambda: qk_proj("q", 1, 2), lambda: qk_proj("k", 1, 2)],
            (2, 1): [lambda: qk_proj("q", 1, 3), lambda: qk_proj("k", 1, 3)],
        }
        fillers.append(lambda: cast_relayout_k_j0(0))
        fillers.extend([(lambda t=t: v_proj(t)) for t in (2, 3)])
        fillers.append(load_b1)
        fillers.append(lambda: cast_relayout_k_j0(1))
        fillers.extend([(lambda t=t: v_proj(t)) for t in (4, 5)])
        fillers.append(load_b2)
        fillers.extend([(lambda t=t: v_proj(t)) for t in (6, 7)])
        fillers.append(load_b3)
        fillers.extend([(lambda t=t: v_proj(t)) for t in range(8, 16)])

        # v0/v1 are needed by the very first avs: emit directly
        v_proj(0)
        v_proj(1)

        osb = {}
        for j5 in range(NJ):
            for p in (0, 1):
                for u in pre_units.get((j5, p), []):
                    u()
                pts = {}
                for KT in range(4 * (j5 + 1)):
                    pull_filler()
                    pts[KT] = scores(p, j5, KT)
                    tl = KT - 4 * j5
                    if tl >= 0:
                        T = KT
                        if p == 0:
                            osb[T] = work.tile(
                                [P, HPC, 64], BF16, tag="osb", bufs=8, name=f"osb{T}"
                            )
                        av(2 * p, j5, tl, pts, osb[T])
                        av(2 * p + 1, j5, tl, pts, osb[T])
                        if p == 1:
                            transpose_pair(0, T, osb[T])
                            transpose_pair(1, T, osb[T])
                            out_proj(T)
        while fillers:
            pull_filler()


def _build_program(split_waits=True):
    nc = bass.Bass("TRN2", target_bir_lowering=False, debug=False, num_devices=NCORES)
    xT = nc.dram_tensor("xT", [ED, CTX], BF16, kind="ExternalInput").ap()
    x8 = nc.dram_tensor("x8", [P, 4, 2, CTX], FP8, kind="ExternalInput").ap()
    w8q = nc.dram_tensor("w8q", [P, 4, 2, 256], FP8, kind="ExternalInput").ap()
    w8k = nc.dram_tensor("w8k", [P, 4, 2, 256], FP8, kind="ExternalInput").ap()
    wv = nc.dram_tensor("wv", [ED, 256], BF16, kind="ExternalInput").ap()
    wo = nc.dram_tensor("wo", [256, ED], BF16, kind="ExternalInput").ap()
    sin2 = nc.dram_tensor("sin2", [P, CTX], BF16, kind="ExternalInput").ap()
    tri2 = nc.dram_tensor("tri2", [P, 2 * P], BF16, kind="ExternalInput").ap()
    ident = nc.dram_tensor("ident", [P, P], BF16, kind="ExternalInput").ap()
    z = nc.dram_tensor("z", [CTX, ED], F32, kind="ExternalOutput").ap()
    with tile.TileContext(nc) as tc:
        _emit(nc, xT, x8, w8q, w8k, wv, wo, sin2, tri2, ident, z, tc)
    if split_waits:
        _split_multi_waits(nc)
    return nc


_PROGRAM = None


def _get_program():
    global _PROGRAM
    if _PROGRAM is None:
        _PROGRAM = _build_program()
    return _PROGRAM


def _host_tables():
    # rotate-half fold matrix: q_rot = R q
    Rm = np.zeros((HD, HD), np.float32)
    for i in range(HD // 2):
        Rm[i, i] = 1.0
        Rm[i, i + 32] = -1.0
        Rm[i + 32, i + 32] = 1.0
        Rm[i + 32, i] = 1.0
    j = np.arange(HD // 2, dtype=np.float32)
    thetas = 1.0 / ROPE_BASE ** (2.0 * j / (HD // 2))
    pos = np.arange(CTX, dtype=np.float32)
    ang = pos[:, None] * thetas[None, :]
    sinT = np.sin(np.concatenate([ang, ang], axis=-1)).T.astype(np.float32)  # [64,CTX]
    sin2 = np.ascontiguousarray(np.tile(sinT, (2, 1))).astype(
        ml_dtypes.bfloat16
    )  # [128, CTX]
    cg = np.arange(P)[None, :]
    ii = np.arange(P)[:, None]
    tri = (cg >= ii).astype(np.float32)  # keep q >= k
    tri2 = np.ascontiguousarray(np.concatenate([tri, tri], axis=1)).astype(
        ml_dtypes.bfloat16
    )  # [128, 256]
    ident = np.eye(P, dtype=np.float32).astype(ml_dtypes.bfloat16)
    return Rm, sin2, tri2, ident


def _pack_dr(a):
    """[1024, n] -> [128, 4, 2, n] fp8 with element e = 256*ci + 128*j + p."""
    n = a.shape[1]
    return np.ascontiguousarray(
        a.reshape(4, 2, P, n).transpose(2, 0, 1, 3)
    ).astype(ml_dtypes.float8_e4m3)


def _run(x, Wq, Wk, Wv, Wo):
    nc = _get_program()
    Rm, sin2, tri2, ident = _host_tables()

    def fold(W):
        W2 = W.reshape(ED, NH, HD)
        return np.einsum("enh,gh->eng", W2, Rm).reshape(ED, NH * HD)

    bf = ml_dtypes.bfloat16
    Wq_f = fold(Wq)
    Wk_f = fold(Wk)
    Wv_b = Wv.astype(bf)
    Wo_b = Wo.astype(bf)
    xT_f = [np.ascontiguousarray(x[b].T) for b in range(BATCH)]
    x8_b = [_pack_dr(t) for t in xT_f]
    xT_b = [t.astype(bf) for t in xT_f]

    in_maps = []
    for core in range(NCORES):
        b, g = core // 4, core % 4
        cs = slice(256 * g, 256 * (g + 1))
        in_maps.append(
            {
                "xT": xT_b[b],
                "x8": x8_b[b],
                "w8q": _pack_dr(np.ascontiguousarray(Wq_f[:, cs])),
                "w8k": _pack_dr(np.ascontiguousarray(Wk_f[:, cs])),
                "wv": np.ascontiguousarray(Wv_b[:, cs]),
                "wo": np.ascontiguousarray(Wo_b[cs, :]),
                "sin2": sin2,
                "tri2": tri2,
                "ident": ident,
            }
        )
    return nc, in_maps


def kernel(x, Wq, Wk, Wv, Wo, bo):
    x = np.asarray(x, dtype=np.float32)
    nc, in_maps = _run(x, np.asarray(Wq, np.float32), np.asarray(Wk, np.float32),
                       np.asarray(Wv, np.float32), np.asarray(Wo, np.float32))
    res = run_bass_kernel_spmd(nc, in_maps, core_ids=list(range(NCORES)))
    out = np.zeros((BATCH, CTX, ED), np.float32)
    for core in range(NCORES):
        b = core // 4
        out[b] += res.results[core]["z"]
    out += np.asarray(bo, np.float32)[None, None, :]
    return out
